# revision 2
# baseline (speedup 1.0000x reference)
"""Causal attention kernel for TRN2, 8 NeuronCores.

Problem: B=4, S=2048, D=1024 single-head causal attention, scale 1/sqrt(64).
  out = softmax_causal((x@Wq+bq) @ (x@Wk+bk)^T / 8) @ (x@Wv+bv) @ Wo + bo

Sharding: 2 cores per batch; query blocks paired odd/even so the uniform SPMD
key-block schedule NKP=[16,14,12,10,8,6,4,2] (72 blocks) is near-ideal (68).
Core A (even) takes odd q-blocks [15,13,...,1] exactly; core B pads one block
per slot, masked via host tiles (WIN=2: diagonal triangle + full/zero).

All matmuls run bf16 (full PE rate at any moving dim; accumulate fp32).
Scores are computed TRANSPOSED (S^T[k,q] = K_blk^T ... via kt stationary),
so exp(S^T) feeds P^T directly as the P@V stationary and the attention
output accumulates as att^T = sum_k V_k^T P^T_k — no PE transposes at all.
Z (softmax denom) comes from a ones-column matmul accumulated beside att^T.
Normalization and biases are folded on the host:
  out = U/Z + (bo + bv@Wo),  U = (unnormalized P V) @ Wo;  bk drops (softmax
  row invariant); bq is applied as per-partition activation bias on Q^T.
"""
import sys
sys.path.insert(0, "/opt/trn_rl_repo")

import numpy as np
from contextlib import ExitStack

import concourse.bacc as bacc
import concourse.mybir as mybir
import concourse.tile as tile

F32 = mybir.dt.float32
BF16 = mybir.dt.bfloat16
EXP = mybir.ActivationFunctionType.Exp
IDENT = mybir.ActivationFunctionType.Identity

B, S, D = 4, 2048, 1024
NB = S // 128             # 16 key/query blocks per batch
QLOC = 1024               # queries per core (8 blocks)
SCHED_A = [15, 13, 11, 9, 7, 5, 3, 1]
SCHED_B = [14, 12, 10, 8, 6, 4, 2, 0]
NKP = [16, 14, 12, 10, 8, 6, 4, 2]   # uniform key-blocks per slot
WIN = 2                              # masked window (last WIN blocks of a slot)
MASKVAL = -1e30

_NC_CACHE = {}


def build_nc(phases=('proj', 'attn')):
    nc = bacc.Bacc("TRN2", target_bir_lowering=False, debug=False, num_devices=8)

    xt = nc.dram_tensor("xt", [D, S], BF16, kind="ExternalInput").ap()      # x^T (this batch)
    xq = nc.dram_tensor("xq", [D, QLOC], BF16, kind="ExternalInput").ap()   # x^T cols of my queries
    wp = nc.dram_tensor("wp", [4, D, D], BF16, kind="ExternalInput").ap()   # Wk,Wv,Wq,Wo
    misc = nc.dram_tensor("misc", [8, 128, 2 * 128 + 1], F32, kind="ExternalInput").ap()
    ud = nc.dram_tensor("ud", [QLOC, D], BF16, kind="ExternalOutput").ap()  # U = P V Wo (unnormalized)
    zd = nc.dram_tensor("zd", [1, QLOC], F32, kind="ExternalOutput").ap()   # Z per query (slot order)

    with tile.TileContext(nc) as tc, ExitStack() as ctx:
        # ---- SBUF pools (per-partition KB in comments; ~208KB usable)
        xt_p = ctx.enter_context(tc.tile_pool(name="xt", bufs=1))      # 8 x 4KB = 32
        kt_p = ctx.enter_context(tc.tile_pool(name="kt", bufs=1))      # 8 x 4KB = 32
        v_p = ctx.enter_context(tc.tile_pool(name="v", bufs=1))        # 16 x 2KB = 32
        qt_p = ctx.enter_context(tc.tile_pool(name="qt", bufs=1))      # 8 x 2KB = 16
        w_p = ctx.enter_context(tc.tile_pool(name="w", bufs=17))       # 17 x 2KB = 34
        asb_p = ctx.enter_context(tc.tile_pool(name="asb", bufs=10))   # 10 x 0.25KB = 2.5
        pt_p = ctx.enter_context(tc.tile_pool(name="pt", bufs=4))      # 4 x 0.25KB = 1
        usb_p = ctx.enter_context(tc.tile_pool(name="usb", bufs=4))    # 4 x 1KB = 4
        const_p = ctx.enter_context(tc.tile_pool(name="const", bufs=1))  # ~8.1
        # ---- PSUM pools (8 banks total)
        ps_main = ctx.enter_context(tc.tile_pool(name="psm", bufs=3, space="PSUM"))   # 3 banks
        ps_sT = ctx.enter_context(tc.tile_pool(name="psT", bufs=2, space="PSUM"))     # 2 banks
        ps_attT = ctx.enter_context(tc.tile_pool(name="psA", bufs=1, space="PSUM"))   # 3 banks

        # ---- constants / small loads
        maskt = const_p.tile([128, 8 * 256], F32, tag="maskt")
        for j in range(8):
            nc.sync.dma_start(maskt[:, j * 256:(j + 1) * 256], misc[j, :, 0:256])
        bq_t = []
        for ec in range(8):
            t = const_p.tile([128, 1], F32, tag=f"bq{ec}", name=f"bqt{ec}")
            nc.sync.dma_start(t[:], misc[ec, :, 256:257])
            bq_t.append(t)
        ones = const_p.tile([128, 1], BF16, tag="ones")
        nc.vector.memset(ones[:], 1.0)
        zsb = const_p.tile([1, QLOC], F32, tag="zsb")

        def load_w(wi):
            ws = []
            for dc in range(8):
                t = w_p.tile([128, D], BF16, tag="w", name="w")
                nc.sync.dma_start(t[:], wp[wi, dc * 128:(dc + 1) * 128, :])
                ws.append(t)
            return ws

        # x^T resident (8 d-chunks), split DMAs for earlier availability
        wk_t = load_w(0)
        xts = []
        for dc in range(8):
            t = xt_p.tile([128, S], BF16, tag=f"xt{dc}", name=f"xt{dc}")
            nc.sync.dma_start(t[:, 0:1024], xt[dc * 128:(dc + 1) * 128, 0:1024])
            nc.sync.dma_start(t[:, 1024:2048], xt[dc * 128:(dc + 1) * 128, 1024:2048])
            xts.append(t)
        xqs = []
        for dc in range(8):
            t = xt_p.tile([128, QLOC], BF16, tag=f"xq{dc}", name=f"xq{dc}")
            nc.sync.dma_start(t[:], xq[dc * 128:(dc + 1) * 128, :])
            xqs.append(t)

        def psum_copy(dst, src, idx):
            (nc.vector.tensor_copy if idx % 2 == 0 else nc.scalar.copy)(dst, src)

        # ---- K projection: kt[ec] = (x @ Wk)^T chunk, [128 e, 2048 s]
        kt = [kt_p.tile([128, S], BF16, tag=f"kt{ec}", name=f"kt{ec}") for ec in range(8)]
        for ec in (range(8) if 'proj' in phases else range(0)):
            for sq in range(4):
                ps = ps_main.tile([128, 512], F32, tag="psm", name="psm")
                for dc in range(8):
                    nc.tensor.matmul(ps[:], wk_t[dc][:, ec * 128:(ec + 1) * 128],
                                     xts[dc][:, sq * 512:(sq + 1) * 512],
                                     start=(dc == 0), stop=(dc == 7))
                psum_copy(kt[ec][:, sq * 512:(sq + 1) * 512], ps[:], ec * 4 + sq)

        # ---- V projection: v[kb] = x @ Wv, [128 s, 1024 e]
        wv_t = load_w(1)
        v = [v_p.tile([128, D], BF16, tag=f"v{kb}", name=f"v{kb}") for kb in range(NB)]
        for kb in (range(NB) if 'proj' in phases else range(0)):
            for eh in range(2):
                ps = ps_main.tile([128, 512], F32, tag="psm", name="psm")
                for dc in range(8):
                    nc.tensor.matmul(ps[:], xts[dc][:, kb * 128:(kb + 1) * 128],
                                     wv_t[dc][:, eh * 512:(eh + 1) * 512],
                                     start=(dc == 0), stop=(dc == 7))
                psum_copy(v[kb][:, eh * 512:(eh + 1) * 512], ps[:], kb * 2 + eh)

        # ---- Q projection (+bq): qt[ec] = (xq @ Wq)^T chunk + bq, [128 e, 1024 q]
        wq_t = load_w(2)
        qt = [qt_p.tile([128, QLOC], BF16, tag=f"qt{ec}", name=f"qt{ec}") for ec in range(8)]
        for ec in (range(8) if 'proj' in phases else range(0)):
            for qh in range(2):
                ps = ps_main.tile([128, 512], F32, tag="psm", name="psm")
                for qi in range(4):
                    for dc in range(8):
                        nc.tensor.matmul(ps[:, qi * 128:(qi + 1) * 128],
                                         wq_t[dc][:, ec * 128:(ec + 1) * 128],
                                         xqs[dc][:, (qh * 4 + qi) * 128:(qh * 4 + qi + 1) * 128],
                                         start=(qi == 0 and dc == 0), stop=(dc == 7))
                nc.scalar.activation(qt[ec][:, qh * 512:(qh + 1) * 512], ps[:],
                                     IDENT, bias=bq_t[ec][:])

        # ---- Wo resident for attention
        wo_t = load_w(3)

        # ---- attention slots (scores transposed: S^T[k,q]; no PE transposes)
        for j in (range(8) if 'attn' in phases else range(0)):
            nkp = NKP[j]
            attT = ps_attT.tile([128, 1536], F32, tag="psA", name="psA")
            # banks: cols 0:512 = att chunks 0-3, 512:1024 = chunks 4-7,
            # 1024:1536 = Z row (start=True only on first touch per bank)
            for kb in range(nkp):
                sT = ps_sT.tile([128, 512], F32, tag="psT", name="psT")
                for dc in range(8):
                    nc.tensor.matmul(sT[:, 0:128], kt[dc][:, kb * 128:(kb + 1) * 128],
                                     qt[dc][:, j * 128:(j + 1) * 128],
                                     start=(dc == 0), stop=(dc == 7))
                if kb >= nkp - WIN:
                    w = kb - (nkp - WIN)
                    nc.vector.tensor_add(sT[:, 0:128], sT[:, 0:128],
                                         maskt[:, j * 256 + w * 128: j * 256 + w * 128 + 128])
                pT = pt_p.tile([128, 128], BF16, tag="pt", name="pt")
                nc.scalar.activation(pT[:], sT[:, 0:128], EXP, scale=0.125)
                for ec in range(8):
                    nc.tensor.matmul(attT[:, ec * 128:(ec + 1) * 128],
                                     v[kb][:, ec * 128:(ec + 1) * 128], pT[:],
                                     start=(kb == 0 and ec % 4 == 0), stop=(kb == nkp - 1))
                nc.tensor.matmul(attT[0:1, 1024:1152], ones[:], pT[:],
                                 start=(kb == 0), stop=(kb == nkp - 1))

            att_sb = []
            for ec in range(8):
                t = asb_p.tile([128, 128], BF16, tag="asb", name="asb")
                psum_copy(t[:], attT[:, ec * 128:(ec + 1) * 128], ec)
                att_sb.append(t)
            nc.vector.tensor_copy(zsb[0:1, j * 128:(j + 1) * 128], attT[0:1, 1024:1152])

            for eh in range(2):
                ops = ps_main.tile([128, 512], F32, tag="psm", name="psm")
                for ec in range(8):
                    nc.tensor.matmul(ops[:], att_sb[ec][:], wo_t[ec][:, eh * 512:(eh + 1) * 512],
                                     start=(ec == 0), stop=(ec == 7))
                usb = usb_p.tile([128, 512], BF16, tag="usb", name="usb")
                psum_copy(usb[:], ops[:], j * 2 + eh)
                nc.sync.dma_start(ud[j * 128:(j + 1) * 128, eh * 512:(eh + 1) * 512], usb[:])

        if 'attn' in phases:
            nc.sync.dma_start(zd, zsb[0:1, :])

    nc.compile()
    return nc


def _host_prep(x, Wq, bq, Wk, bk, Wv, bv, Wo, bo):
    """Build the 8 per-core input maps (bf16 x/weights, f32 misc)."""
    bf16 = mybir.dt.np(BF16)
    wpack = np.stack([Wk, Wv, Wq, Wo]).astype(bf16)
    # masks in TRANSPOSED (k, q) layout: visible iff global_k <= global_q
    ql, kl = np.meshgrid(np.arange(128), np.arange(128), indexing='xy')
    # kl[r,c] = r (k_local), ql[r,c] = c (q_local)
    tri = np.where(np.arange(128)[:, None] > np.arange(128)[None, :],
                   MASKVAL, 0.0).astype(np.float32)       # mask k_local > q_local
    full = np.full((128, 128), MASKVAL, np.float32)
    zero = np.zeros((128, 128), np.float32)

    in_maps = []
    for core in range(8):
        b = core // 2
        sched = SCHED_A if core % 2 == 0 else SCHED_B
        xtb = np.ascontiguousarray(x[b].T).astype(bf16)                  # [D, S]
        xqb = np.ascontiguousarray(
            np.concatenate([x[b].T[:, g * 128:(g + 1) * 128] for g in sched], axis=1)
        ).astype(bf16)
        misc = np.zeros((8, 128, 257), np.float32)
        for j, g in enumerate(sched):
            for w in range(WIN):
                kb = NKP[j] - WIN + w
                if kb < g:
                    m = zero
                elif kb == g:
                    m = tri
                else:
                    m = full
                misc[j, :, w * 128:(w + 1) * 128] = m
        misc[:, :, 256] = bq.reshape(8, 128)
        in_maps.append({"xt": xtb, "xq": xqb, "wp": wpack, "misc": misc})
    return in_maps


def _make_runner(nc, n_cores=8):
    """Persistent jitted PJRT runner (one trace+compile per process)."""
    import jax
    from jax.sharding import Mesh, PartitionSpec, NamedSharding
    from jax.experimental.shard_map import shard_map
    from concourse import bass2jax
    from concourse.bass2jax import _bass_exec_p, install_neuronx_cc_hook

    install_neuronx_cc_hook()
    pname = nc.partition_id_tensor.name if nc.partition_id_tensor else None
    in_names, out_names, out_avals = [], [], []
    for alloc in nc.m.functions[0].allocations:
        if not isinstance(alloc, mybir.MemoryLocationSet):
            continue
        name = alloc.memorylocations[0].name
        if alloc.kind == "ExternalInput":
            if name != pname:
                in_names.append(name)
        elif alloc.kind == "ExternalOutput":
            out_names.append(name)
            out_avals.append(jax.core.ShapedArray(tuple(alloc.tensor_shape),
                                                  mybir.dt.np(alloc.dtype)))
    n_params, n_outs = len(in_names), len(out_avals)
    all_names = in_names + out_names + ([pname] if pname else [])

    def _body(*args):
        operands = list(args)
        if pname is not None:
            operands.append(bass2jax.partition_id_tensor())
        outs = _bass_exec_p.bind(
            *operands,
            out_avals=tuple(out_avals),
            in_names=tuple(all_names),
            out_names=tuple(out_names),
            lowering_input_output_aliases=(),
            sim_require_finite=True,
            sim_require_nnan=True,
            nc=nc,
        )
        return tuple(outs)

    devices = jax.devices()[:n_cores]
    mesh = Mesh(np.asarray(devices), ("core",))
    in_specs = (PartitionSpec("core"),) * (n_params + n_outs)
    out_specs = (PartitionSpec("core"),) * n_outs
    fn = jax.jit(shard_map(_body, mesh=mesh, in_specs=in_specs, out_specs=out_specs,
                           check_rep=False),
                 donate_argnums=tuple(range(n_params, n_params + n_outs)),
                 keep_unused=True)
    shard = NamedSharding(mesh, PartitionSpec("core"))

    def run(in_maps):
        conc = [np.concatenate([np.asarray(in_maps[c][n]) for c in range(n_cores)],
                               axis=0) for n in in_names]
        dev_in = [jax.device_put(a, shard) for a in conc]
        zb = [jax.device_put(np.zeros((n_cores * a.shape[0], *a.shape[1:]), a.dtype),
                             shard) for a in out_avals]
        outs = fn(*dev_in, *zb)
        host = [np.asarray(o) for o in outs]
        return [{n: host[i].reshape(n_cores, *out_avals[i].shape)[c]
                 for i, n in enumerate(out_names)} for c in range(n_cores)]

    return run


def kernel(x, Wq, bq, Wk, bk, Wv, bv, Wo, bo):
    x = np.asarray(x, np.float32)
    args = [np.asarray(a, np.float32) for a in (Wq, bq, Wk, bk, Wv, bv, Wo, bo)]
    Wq, bq, Wk, bk, Wv, bv, Wo, bo = args

    if "run" not in _NC_CACHE:
        _NC_CACHE["nc"] = build_nc()
        _NC_CACHE["run"] = _make_runner(_NC_CACHE["nc"])

    in_maps = _host_prep(x, Wq, bq, Wk, bk, Wv, bv, Wo, bo)
    results = _NC_CACHE["run"](in_maps)

    bop = (bo.astype(np.float64) + bv.astype(np.float64) @ Wo.astype(np.float64)).astype(np.float32)
    out = np.empty((B, S, D), np.float32)
    for core in range(8):
        b = core // 2
        sched = SCHED_A if core % 2 == 0 else SCHED_B
        u = np.asarray(results[core]["ud"], np.float32)          # [QLOC, D]
        z = np.asarray(results[core]["zd"], np.float32).reshape(QLOC)
        o = u / z[:, None] + bop[None, :]
        for j, g in enumerate(sched):
            out[b, g * 128:(g + 1) * 128, :] = o[j * 128:(j + 1) * 128, :]
    return out
